# revision 15
# baseline (speedup 1.0000x reference)
"""BinaryConnect dense layer on 8 Trainium2 NeuronCores.

Computes Y = X @ sign(W) + bias for X[8192,4096], W[4096,4096] f32.

Strategy (data-parallel over X rows, 1024 rows/core), mode "dr":
- sign(W) in {-1,+1} is exact in fp8 e4m3, so the matmul can run in the
  PE's DoubleRow fp8 perf mode: 2 fp8 weights per cell, contraction of
  256 per pass, ~2x the bf16/fp16 matmul rate.
- X is split on HOST into hi = e4m3(x) and lo = e4m3(x - hi). The hi
  part alone gives rel err ~2.6e-2 (past the 2e-2 gate), so the first
  L_LO=20 of 32 k-tiles also get a lo-correction pass, bringing rel err
  to ~1.5e-2 (~25% margin; HW matches the numpy sim bit-for-bit). The
  lo passes REUSE the same loaded sign weights as the hi passes (the
  signs are identical), so LDWEIGHTS count stays at the hi-only level:
  16 weight loads per m-tile, 4 MMs each for corrected k-pairs.
- Measured: MM issue-to-issue is 216ns (512 output cols at 1 col/cycle,
  2.4GHz); DoubleRow doubles contraction per pass, not column rate, so
  time = passes * 216ns. 26 passes/m-tile-chunk * 64 = 1664 MMs =
  ~359us + ~26us fixed runtime overhead -> ~385us (baseline fp16:
  498us). int8/uint8 matmuls and DoublePixel/DoubleColumn modes are
  rejected or silently ignored by this compiler, so fp8 DoubleRow is
  the only >1x PE rate available.
- The chip sometimes sits in a ~2.0GHz power state (259ns/MM -> ~458us
  total); that is environmental hysteresis, not a kernel property —
  identical code measures 385us once the clock returns to 2.4GHz.
  Deliberately keeping the PE warm with dummy matmuls made this WORSE
  (sustained draw trips the downclock); don't re-add warm-up.
- W is binarized on host straight to e4m3 bytes (0x38/0xB8) and packed
  in per-m-tile contiguous order, so the device streams 16MB (not 64MB
  f32) and runs no DVE work at all.
- Each core computes Y_shard^T; host transposes/concatenates (layout
  only).

Mode "fp16" (fallback): original single-pass fp16 kernel, ~498us.
"""

import numpy as np
import ml_dtypes

import concourse.bass as bass
import concourse.mybir as mybir
from concourse import bacc
from concourse.tile import TileContext
from concourse.bass_utils import run_bass_kernel_spmd

P = 128
N_CORES = 8
N_FULL = 8192
K_DIM = 4096
M_DIM = 4096
MODE = "dr"
L_LO = 20  # k-tiles (of 32) that get the lo-correction pass

FP8 = ml_dtypes.float8_e4m3


def build_dr_program(n_rows, k_dim, m_dim, l_lo=L_LO, n_free=512):
    """One-core SPMD program: yt[m, n] = (sign(W).T @ x) + b via fp8 DoubleRow.

    DRAM params (host-packed, all fp8 e4m3 unless noted):
      xhi [P, KT, n]         hi  = e4m3(x^T), k-tile-major
      xlo [P, l_lo, n]       lo  = e4m3(x^T - hi) for k-tiles < l_lo
      wpk [P, MT, KP, 2, P]  sign(W) packed per (m-tile, k-pair, slot)
      b   [m, 1] f32
      yt  [m, n] f32 out
    """
    f32 = mybir.dt.float32
    fp8 = mybir.dt.float8e4
    DR = mybir.MatmulPerfMode.DoubleRow
    KT = k_dim // P            # 32 k-tiles
    KP = KT // 2               # 16 k-pairs (DoubleRow contraction 256)
    LP = l_lo // 2             # k-pairs with lo correction
    NMT = m_dim // P           # 32 m-tiles
    nchunks = n_rows // n_free # 2 chunks of 512 rows
    XB = 4                     # k-tiles per X DMA batch

    nc = bacc.Bacc()
    xhi = nc.declare_dram_parameter("xhi", [P, KT, n_rows], fp8, isOutput=False)
    xlo = nc.declare_dram_parameter("xlo", [P, l_lo, n_rows], fp8, isOutput=False)
    wpk = nc.declare_dram_parameter("wpk", [P, NMT, KP, 2, P], fp8, isOutput=False)
    b = nc.declare_dram_parameter("b", [m_dim, 1], f32, isOutput=False)
    yt = nc.declare_dram_parameter("yt", [m_dim, n_rows], f32, isOutput=True)

    b_r = b.ap().rearrange("(mt p) o -> p mt o", p=P)

    GRP0 = 2  # startup m-tiles accumulated k-major while X streams in

    with TileContext(nc) as tc:
        with (
            tc.tile_pool(name="xres", bufs=1) as xres_pool,
            tc.tile_pool(name="wt", bufs=4) as wt_pool,
            tc.tile_pool(name="biasp", bufs=1) as bias_pool,
            tc.tile_pool(name="outp", bufs=3) as out_pool,
            tc.tile_pool(name="psump", bufs=1, space="PSUM") as psum_pool,
        ):
            # All 8 PSUM banks under explicit tags: startup m-tiles use
            # banks 0-3; the main loop cycles pairs starting at (4,5) for
            # a 4-deep m-tile pipeline.
            def psum_pair(b0, b1):
                return [
                    psum_pool.tile([P, n_free], f32, name=f"ps{b}", tag=f"ps{b}")
                    for b in (b0, b1)
                ]

            bts = bias_pool.tile([P, NMT, 1], f32, name="bts", tag="bts")
            nc.sync.dma_start(out=bts[:], in_=b_r[:, :, :])

            def load_wtile(mt):
                wt = wt_pool.tile([P, KP, 2, P], fp8, name="wt", tag="wt")
                nc.sync.dma_start(out=wt[:], in_=wpk.ap()[:, mt, :, :, :])
                return wt

            # First weight tile, then the first X k-pair, before anything
            # else: MM#0 needs exactly wt0 + xhi/xlo k-tiles 0-1.
            wtiles = {0: load_wtile(0)}

            # Resident X: DMA straight into SBUF, interleaving hi/lo
            # batches so the k-prefix completes early for the PE chase.
            # The first k-pair ships as its own small batch so MM#0 can
            # issue as early as possible.
            xhi_t = xres_pool.tile([P, KT, n_rows], fp8, name="xhi", tag="xhi")
            xlo_t = xres_pool.tile([P, l_lo, n_rows], fp8, name="xlo", tag="xlo")
            bounds = [0, 2] + [XB * i for i in range(1, KT // XB + 1)]
            for bi, (k0, ke) in enumerate(zip(bounds[:-1], bounds[1:])):
                nc.sync.dma_start(
                    out=xhi_t[:, k0:ke, :], in_=xhi.ap()[:, k0:ke, :]
                )
                if k0 < l_lo:
                    le = min(ke, l_lo)
                    nc.sync.dma_start(
                        out=xlo_t[:, k0:le, :], in_=xlo.ap()[:, k0:le, :]
                    )
                if bi == 0:
                    for mt in range(1, GRP0):
                        wtiles[mt] = load_wtile(mt)

            wtiles[GRP0] = load_wtile(GRP0)

            def mm_group(psums_mt, wts, g):
                # All MMs for k-pair g across the m-tiles of this group,
                # reusing each loaded weight for hi + lo passes.
                for mi, wt in enumerate(wts):
                    lhsT = wt[:, g, :, :]
                    for j in range(nchunks):
                        nc.tensor.matmul(
                            psums_mt[mi][j][:],
                            lhsT,
                            xhi_t[:, 2 * g:2 * g + 2, j * n_free:(j + 1) * n_free],
                            start=(g == 0),
                            stop=(g == KP - 1),
                            perf_mode=DR,
                        )
                    if g < LP:
                        for j in range(nchunks):
                            nc.tensor.matmul(
                                psums_mt[mi][j][:],
                                lhsT,
                                xlo_t[:, 2 * g:2 * g + 2, j * n_free:(j + 1) * n_free],
                                start=False,
                                stop=False,
                                perf_mode=DR,
                            )

            def evict(mt, psums):
                # Per-chunk ACT + DMA so the output transfer of chunk j
                # overlaps the eviction of chunk j+1 (matters for the tail).
                out_t = out_pool.tile([P, n_rows], f32, name="out_t", tag="out_t")
                for j in range(nchunks):
                    nc.scalar.activation(
                        out=out_t[:, j * n_free:(j + 1) * n_free],
                        in_=psums[j][:],
                        func=mybir.ActivationFunctionType.Identity,
                        bias=bts[:, mt, :],
                        scale=1.0,
                    )
                    nc.sync.dma_start(
                        out=yt[mt * P:(mt + 1) * P, j * n_free:(j + 1) * n_free],
                        in_=out_t[:, j * n_free:(j + 1) * n_free],
                    )

            # Startup group: GRP0 m-tiles k-major in lockstep, so the PE has
            # GRP0*4 matmuls per arriving X k-pair and never starves on the
            # inbound X DMA stream.
            ps0 = [psum_pair(2 * mi, 2 * mi + 1) for mi in range(GRP0)]
            wts0 = [wtiles.pop(mt) for mt in range(GRP0)]
            for g in range(KP):
                mm_group(ps0, wts0, g)
            for mi in range(GRP0):
                evict(mi, ps0[mi])

            # Main loop: one m-tile at a time, weights prefetched one
            # ahead, PSUM bank pairs cycling (6,7),(0,1),(2,3),(4,5).
            main_pairs = [(4, 5), (6, 7), (0, 1), (2, 3)]
            for mt in range(GRP0, NMT):
                wt = wtiles.pop(mt)
                if mt + 1 < NMT and mt + 1 not in wtiles:
                    wtiles[mt + 1] = load_wtile(mt + 1)
                psums = psum_pair(*main_pairs[(mt - GRP0) % 4])
                for g in range(KP):
                    mm_group([psums], [wt], g)
                evict(mt, psums)
    nc.compile()
    return nc


def build_bc_program(n_rows, k_dim, m_dim, mb_cols=256, n_free=512, mode="fp16",
                     kb=4, wb_bufs=4):
    """Fallback one-core program: yt = (xt.T @ sign(w)).T + b, fp16 single pass."""
    f32 = mybir.dt.float32
    f32r = mybir.dt.float32r
    bf16 = mybir.dt.bfloat16
    fp16 = mybir.dt.float16
    if mode == "fp32r":
        mb_cols = min(mb_cols, P)
    if mode == "hilo":
        wb_bufs = 2
        kb = min(kb, 2)
    KT = k_dim // P
    kb = min(kb, KT)
    KB_N = KT // kb
    MTPB = mb_cols // P
    NMB = m_dim // mb_cols
    nchunks = (n_rows + n_free - 1) // n_free
    chunk_sz = [min(n_free, n_rows - j * n_free) for j in range(nchunks)]
    GRP = 1

    nc = bacc.Bacc()
    xt = nc.declare_dram_parameter("xt", [k_dim, n_rows], f32, isOutput=False)
    w = nc.declare_dram_parameter("w", [k_dim, m_dim], f32, isOutput=False)
    b = nc.declare_dram_parameter("b", [m_dim, 1], f32, isOutput=False)
    yt = nc.declare_dram_parameter("yt", [m_dim, n_rows], f32, isOutput=True)

    wb_dt = {"hilo": bf16, "fp32r": f32r, "fp16": fp16}[mode]
    xt_r = xt.ap().rearrange("(kt p) n -> p kt n", p=P)
    w_r = w.ap().rearrange("(kt p) m -> p kt m", p=P)
    b_r = b.ap().rearrange("(mt p) o -> p mt o", p=P)

    with TileContext(nc) as tc:
        with (
            tc.tile_pool(name="xstage", bufs=2) as xstage_pool,
            tc.tile_pool(name="xres", bufs=1) as xres_pool,
            tc.tile_pool(name="wstage", bufs=2 if mode == "fp16" else 3) as wstage_pool,
            tc.tile_pool(name="wbp", bufs=wb_bufs) as wb_pool,
            tc.tile_pool(name="biasp", bufs=1) as bias_pool,
            tc.tile_pool(name="outp", bufs=3) as out_pool,
            tc.tile_pool(
                name="psump",
                bufs=max(1, 8 // (GRP * MTPB * nchunks)),
                space="PSUM",
            ) as psum_pool,
        ):
            bts = bias_pool.tile([P, m_dim // P, 1], f32, name="bts", tag="bts")
            nc.sync.dma_start(out=bts[:], in_=b_r[:, :, :])

            def produce_wb_group(g):
                mbs = [g * GRP + i for i in range(GRP)]
                wbs = [
                    wb_pool.tile([P, KT * mb_cols], wb_dt, name="wb", tag="wb")
                    for _ in mbs
                ]
                for kg in range(KB_N):
                    for i, mb in enumerate(mbs):
                        wf = wstage_pool.tile(
                            [P, kb, mb_cols], f32, name="wf", tag="wf"
                        )
                        nc.sync.dma_start(
                            out=wf[:],
                            in_=w_r[:, kg * kb:(kg + 1) * kb,
                                    mb * mb_cols:(mb + 1) * mb_cols],
                        )
                        for t in range(kb):
                            k = kg * kb + t
                            nc.vector.tensor_scalar(
                                out=wbs[i][:, k * mb_cols:(k + 1) * mb_cols],
                                in0=wf[:, t, :],
                                scalar1=0.0,
                                scalar2=0.5,
                                op0=mybir.AluOpType.is_ge,
                                op1=mybir.AluOpType.subtract,
                            )
                return wbs

            NGRP = NMB // GRP
            wb_tiles = {0: produce_wb_group(0)}

            if mode == "hilo":
                xhi = xres_pool.tile([P, KT * n_rows], bf16, name="xhi", tag="xhi")
                xlo = xres_pool.tile([P, KT * n_rows], bf16, name="xlo", tag="xlo")
                for kg in range(KB_N):
                    xf = xstage_pool.tile([P, kb, n_rows], f32, name="xf", tag="xf")
                    nc.sync.dma_start(
                        out=xf[:], in_=xt_r[:, kg * kb:(kg + 1) * kb, :]
                    )
                    for t in range(kb):
                        k = kg * kb + t
                        hi = xhi[:, k * n_rows:(k + 1) * n_rows]
                        lo = xlo[:, k * n_rows:(k + 1) * n_rows]
                        nc.vector.tensor_copy(out=hi, in_=xf[:, t, :])
                        nc.vector.tensor_sub(out=lo, in0=xf[:, t, :], in1=hi)
            else:
                x_dt = f32r if mode == "fp32r" else fp16
                xall = xres_pool.tile([P, KT * n_rows], x_dt, name="xall", tag="xall")
                for kg in range(KB_N):
                    xf = xstage_pool.tile([P, kb, n_rows], f32, name="xf", tag="xf")
                    nc.sync.dma_start(
                        out=xf[:], in_=xt_r[:, kg * kb:(kg + 1) * kb, :]
                    )
                    for t in range(kb):
                        k = kg * kb + t
                        nc.scalar.copy(
                            out=xall[:, k * n_rows:(k + 1) * n_rows],
                            in_=xf[:, t, :],
                        )

            if NGRP > 1:
                wb_tiles[1] = produce_wb_group(1)

            for g in range(NGRP):
                wbs = wb_tiles.pop(g)
                if g + 1 < NGRP and g + 1 not in wb_tiles:
                    wb_tiles[g + 1] = produce_wb_group(g + 1)

                psums = [
                    [
                        psum_pool.tile(
                            [P, chunk_sz[j]], f32,
                            name=f"ps{mi}_{j}", tag=f"ps{mi}_{j}",
                        )
                        for j in range(nchunks)
                    ]
                    for mi in range(GRP * MTPB)
                ]
                for k in range(KT):
                    for mi in range(GRP * MTPB):
                        wb = wbs[mi // MTPB]
                        mw = mi % MTPB
                        lhsT = wb[:, k * mb_cols + mw * P:k * mb_cols + (mw + 1) * P]
                        if mode == "hilo":
                            for j in range(nchunks):
                                c0 = k * n_rows + j * n_free
                                rh = xhi[:, c0:c0 + chunk_sz[j]]
                                rl = xlo[:, c0:c0 + chunk_sz[j]]
                                nc.tensor.matmul(
                                    psums[mi][j][:], lhsT, rh,
                                    start=(k == 0), stop=False,
                                )
                                nc.tensor.matmul(
                                    psums[mi][j][:], lhsT, rl,
                                    start=False, stop=(k == KT - 1),
                                )
                        else:
                            for j in range(nchunks):
                                c0 = k * n_rows + j * n_free
                                rr = xall[:, c0:c0 + chunk_sz[j]]
                                nc.tensor.matmul(
                                    psums[mi][j][:], lhsT, rr,
                                    start=(k == 0), stop=(k == KT - 1),
                                )
                for mi in range(GRP * MTPB):
                    m = g * GRP * MTPB + mi
                    out_t = out_pool.tile([P, n_rows], f32, name="out_t", tag="out_t")
                    for j in range(nchunks):
                        nc.scalar.activation(
                            out=out_t[:, j * n_free:j * n_free + chunk_sz[j]],
                            in_=psums[mi][j][:],
                            func=mybir.ActivationFunctionType.Identity,
                            bias=bts[:, m, :],
                            scale=2.0,
                        )
                    nc.sync.dma_start(out=yt[m * P:(m + 1) * P, :], in_=out_t[:])
    nc.compile()
    return nc


_NC_CACHE = {}


def _get_program(mode=None):
    if mode is None:
        mode = MODE
    key = (N_FULL // N_CORES, K_DIM, M_DIM, mode)
    if key not in _NC_CACHE:
        if mode == "dr":
            _NC_CACHE[key] = build_dr_program(*key[:3])
        else:
            _NC_CACHE[key] = build_bc_program(*key[:3], mode=mode)
    return _NC_CACHE[key]


def _pack_w_dr(w):
    """sign(W) -> e4m3 bytes packed [P, MT, KP, 2, P] (per-m-tile contiguous)."""
    KT = K_DIM // P
    s8 = np.where(np.asarray(w, dtype=np.float32) >= 0, 0x38, 0xB8).astype(np.uint8)
    # k = (g*2 + s)*P + p, m = mt*P + mc
    s8 = s8.reshape(KT // 2, 2, P, M_DIM // P, P)       # [g, s, p, mt, mc]
    s8 = np.ascontiguousarray(s8.transpose(2, 3, 0, 1, 4))  # [p, mt, g, s, mc]
    return s8.view(FP8)


def _pack_x_dr(shard):
    """x shard [n, K] f32 -> (xhi [P, KT, n], xlo [P, L_LO, n]) e4m3."""
    n = shard.shape[0]
    KT = K_DIM // P
    hi = shard.astype(FP8)
    lo = (shard - hi.astype(np.float32))[:, :L_LO * P].astype(FP8)

    def to_tiles(a, kt):
        # [n, kt*P] -> [P, kt, n]
        return np.ascontiguousarray(
            a.T.reshape(kt, P, n).transpose(1, 0, 2)
        )

    return to_tiles(hi, KT), to_tiles(lo, L_LO)


def make_in_maps(x, w, b, mode=None):
    if mode is None:
        mode = MODE
    rows = x.shape[0] // N_CORES
    b = np.ascontiguousarray(np.asarray(b, dtype=np.float32).reshape(-1, 1))
    in_maps = []
    if mode == "dr":
        wpk = _pack_w_dr(w)
        for c in range(N_CORES):
            shard = np.asarray(x[c * rows:(c + 1) * rows, :], dtype=np.float32)
            xhi, xlo = _pack_x_dr(shard)
            in_maps.append({"xhi": xhi, "xlo": xlo, "wpk": wpk, "b": b})
    else:
        w = np.ascontiguousarray(np.asarray(w, dtype=np.float32))
        for c in range(N_CORES):
            shard = np.ascontiguousarray(
                np.asarray(x[c * rows:(c + 1) * rows, :], dtype=np.float32).T
            )
            in_maps.append({"xt": shard, "w": w, "b": b})
    return in_maps


def assemble_output(results, n_full=N_FULL, m_dim=M_DIM):
    rows = n_full // N_CORES
    y = np.empty((n_full, m_dim), dtype=np.float32)
    for c in range(N_CORES):
        y[c * rows:(c + 1) * rows, :] = results[c]["yt"].T
    return y


def kernel(x, kernel, bias):
    nc = _get_program()
    in_maps = make_in_maps(x, kernel, bias)
    res = run_bass_kernel_spmd(nc, in_maps, list(range(N_CORES)))
    return assemble_output(res.results)


# revision 16
# speedup vs baseline: 1.0030x; 1.0030x over previous
"""BinaryConnect dense layer on 8 Trainium2 NeuronCores.

Computes Y = X @ sign(W) + bias for X[8192,4096], W[4096,4096] f32.

Strategy (data-parallel over X rows, 1024 rows/core), mode "dr":
- sign(W) in {-1,+1} is exact in fp8 e4m3, so the matmul can run in the
  PE's DoubleRow fp8 perf mode: 2 fp8 weights per cell, contraction of
  256 per pass, ~2x the bf16/fp16 matmul rate.
- X is split on HOST into hi = e4m3(x) and lo = e4m3(x - hi). The hi
  part alone gives rel err ~2.6e-2 (past the 2e-2 gate), so the first
  L_LO=20 of 32 k-tiles also get a lo-correction pass, bringing rel err
  to ~1.5e-2 (~25% margin; HW matches the numpy sim bit-for-bit). The
  lo passes REUSE the same loaded sign weights as the hi passes (the
  signs are identical), so LDWEIGHTS count stays at the hi-only level:
  16 weight loads per m-tile, 4 MMs each for corrected k-pairs.
- Measured: MM issue-to-issue is 216ns (512 output cols at 1 col/cycle,
  2.4GHz); DoubleRow doubles contraction per pass, not column rate, so
  time = passes * 216ns. 26 passes/m-tile-chunk * 64 = 1664 MMs =
  ~359us + ~26us fixed runtime overhead -> ~385us (baseline fp16:
  498us). int8/uint8 matmuls and DoublePixel/DoubleColumn modes are
  rejected or silently ignored by this compiler, so fp8 DoubleRow is
  the only >1x PE rate available.
- The chip sometimes sits in a ~2.0GHz power state (259ns/MM -> ~458us
  total); that is environmental hysteresis, not a kernel property —
  identical code measures 385us once the clock returns to 2.4GHz.
  Deliberately keeping the PE warm with dummy matmuls made this WORSE
  (sustained draw trips the downclock); don't re-add warm-up.
- W is binarized on host straight to e4m3 bytes (0x38/0xB8) and packed
  in per-m-tile contiguous order, so the device streams 16MB (not 64MB
  f32) and runs no DVE work at all.
- Each core computes Y_shard^T; host transposes/concatenates (layout
  only).

Mode "fp16" (fallback): original single-pass fp16 kernel, ~498us.
"""

import numpy as np
import ml_dtypes

import concourse.bass as bass
import concourse.mybir as mybir
from concourse import bacc
from concourse.tile import TileContext
from concourse.bass_utils import run_bass_kernel_spmd

P = 128
N_CORES = 8
N_FULL = 8192
K_DIM = 4096
M_DIM = 4096
MODE = "dr"
L_LO = 20  # k-tiles (of 32) that get the lo-correction pass

FP8 = ml_dtypes.float8_e4m3


def build_dr_program(n_rows, k_dim, m_dim, l_lo=L_LO, n_free=512):
    """One-core SPMD program: yt[m, n] = (sign(W).T @ x) + b via fp8 DoubleRow.

    DRAM params (host-packed, all fp8 e4m3 unless noted):
      xhi [P, KT, n]         hi  = e4m3(x^T), k-tile-major
      xlo [P, l_lo, n]       lo  = e4m3(x^T - hi) for k-tiles < l_lo
      wpk [P, MT, KP, 2, P]  sign(W) packed per (m-tile, k-pair, slot)
      b   [m, 1] f32
      yt  [m, n] f32 out
    """
    f32 = mybir.dt.float32
    fp8 = mybir.dt.float8e4
    DR = mybir.MatmulPerfMode.DoubleRow
    KT = k_dim // P            # 32 k-tiles
    KP = KT // 2               # 16 k-pairs (DoubleRow contraction 256)
    LP = l_lo // 2             # k-pairs with lo correction
    NMT = m_dim // P           # 32 m-tiles
    nchunks = n_rows // n_free # 2 chunks of 512 rows
    XB = 4                     # k-tiles per X DMA batch

    nc = bacc.Bacc()
    xhi = nc.declare_dram_parameter("xhi", [P, KT, n_rows], fp8, isOutput=False)
    xlo = nc.declare_dram_parameter("xlo", [P, l_lo, n_rows], fp8, isOutput=False)
    wpk = nc.declare_dram_parameter("wpk", [P, NMT, KP, 2, P], fp8, isOutput=False)
    b = nc.declare_dram_parameter("b", [m_dim, 1], f32, isOutput=False)
    yt = nc.declare_dram_parameter("yt", [m_dim, n_rows], f32, isOutput=True)

    b_r = b.ap().rearrange("(mt p) o -> p mt o", p=P)

    GRP0 = 2  # startup m-tiles accumulated k-major while X streams in

    with TileContext(nc) as tc:
        with (
            tc.tile_pool(name="xres", bufs=1) as xres_pool,
            tc.tile_pool(name="wt", bufs=4) as wt_pool,
            tc.tile_pool(name="biasp", bufs=1) as bias_pool,
            tc.tile_pool(name="outp", bufs=3) as out_pool,
            tc.tile_pool(name="psump", bufs=1, space="PSUM") as psum_pool,
        ):
            # All 8 PSUM banks under explicit tags: startup m-tiles use
            # banks 0-3; the main loop cycles pairs starting at (4,5) for
            # a 4-deep m-tile pipeline.
            def psum_pair(b0, b1):
                return [
                    psum_pool.tile([P, n_free], f32, name=f"ps{b}", tag=f"ps{b}")
                    for b in (b0, b1)
                ]

            bts = bias_pool.tile([P, NMT, 1], f32, name="bts", tag="bts")
            nc.sync.dma_start(out=bts[:], in_=b_r[:, :, :])

            def load_wtile(mt, split=False):
                wt = wt_pool.tile([P, KP, 2, P], fp8, name="wt", tag="wt")
                if split:
                    # Startup tiles: ship the first two k-pairs separately so
                    # the PE's first matmuls wait on 64KB, not 512KB.
                    nc.sync.dma_start(
                        out=wt[:, 0:2, :, :], in_=wpk.ap()[:, mt, 0:2, :, :]
                    )
                    nc.sync.dma_start(
                        out=wt[:, 2:, :, :], in_=wpk.ap()[:, mt, 2:, :, :]
                    )
                else:
                    nc.sync.dma_start(out=wt[:], in_=wpk.ap()[:, mt, :, :, :])
                return wt

            # First weight tile, then the first X k-pair, before anything
            # else: MM#0 needs exactly wt0 + xhi/xlo k-tiles 0-1.
            wtiles = {0: load_wtile(0, split=True)}

            # Resident X: DMA straight into SBUF, interleaving hi/lo
            # batches so the k-prefix completes early for the PE chase.
            # The first k-pair ships as its own small batch so MM#0 can
            # issue as early as possible.
            xhi_t = xres_pool.tile([P, KT, n_rows], fp8, name="xhi", tag="xhi")
            xlo_t = xres_pool.tile([P, l_lo, n_rows], fp8, name="xlo", tag="xlo")
            bounds = [0, 2] + [XB * i for i in range(1, KT // XB + 1)]
            for bi, (k0, ke) in enumerate(zip(bounds[:-1], bounds[1:])):
                nc.sync.dma_start(
                    out=xhi_t[:, k0:ke, :], in_=xhi.ap()[:, k0:ke, :]
                )
                if k0 < l_lo:
                    le = min(ke, l_lo)
                    nc.sync.dma_start(
                        out=xlo_t[:, k0:le, :], in_=xlo.ap()[:, k0:le, :]
                    )
                if bi == 0:
                    for mt in range(1, GRP0):
                        wtiles[mt] = load_wtile(mt, split=True)

            wtiles[GRP0] = load_wtile(GRP0)

            def mm_group(psums_mt, wts, g):
                # All MMs for k-pair g across the m-tiles of this group,
                # reusing each loaded weight for hi + lo passes.
                for mi, wt in enumerate(wts):
                    lhsT = wt[:, g, :, :]
                    for j in range(nchunks):
                        nc.tensor.matmul(
                            psums_mt[mi][j][:],
                            lhsT,
                            xhi_t[:, 2 * g:2 * g + 2, j * n_free:(j + 1) * n_free],
                            start=(g == 0),
                            stop=(g == KP - 1),
                            perf_mode=DR,
                        )
                    if g < LP:
                        for j in range(nchunks):
                            nc.tensor.matmul(
                                psums_mt[mi][j][:],
                                lhsT,
                                xlo_t[:, 2 * g:2 * g + 2, j * n_free:(j + 1) * n_free],
                                start=False,
                                stop=False,
                                perf_mode=DR,
                            )

            def evict(mt, psums):
                # Per-chunk ACT + DMA so the output transfer of chunk j
                # overlaps the eviction of chunk j+1 (matters for the tail).
                out_t = out_pool.tile([P, n_rows], f32, name="out_t", tag="out_t")
                for j in range(nchunks):
                    nc.scalar.activation(
                        out=out_t[:, j * n_free:(j + 1) * n_free],
                        in_=psums[j][:],
                        func=mybir.ActivationFunctionType.Identity,
                        bias=bts[:, mt, :],
                        scale=1.0,
                    )
                    nc.sync.dma_start(
                        out=yt[mt * P:(mt + 1) * P, j * n_free:(j + 1) * n_free],
                        in_=out_t[:, j * n_free:(j + 1) * n_free],
                    )

            # Startup group: GRP0 m-tiles k-major in lockstep, so the PE has
            # GRP0*4 matmuls per arriving X k-pair and never starves on the
            # inbound X DMA stream.
            ps0 = [psum_pair(2 * mi, 2 * mi + 1) for mi in range(GRP0)]
            wts0 = [wtiles.pop(mt) for mt in range(GRP0)]
            for g in range(KP):
                mm_group(ps0, wts0, g)
            for mi in range(GRP0):
                evict(mi, ps0[mi])

            # Main loop: one m-tile at a time, weights prefetched one
            # ahead, PSUM bank pairs cycling (6,7),(0,1),(2,3),(4,5).
            main_pairs = [(4, 5), (6, 7), (0, 1), (2, 3)]
            for mt in range(GRP0, NMT):
                wt = wtiles.pop(mt)
                if mt + 1 < NMT and mt + 1 not in wtiles:
                    wtiles[mt + 1] = load_wtile(mt + 1)
                psums = psum_pair(*main_pairs[(mt - GRP0) % 4])
                for g in range(KP):
                    mm_group([psums], [wt], g)
                evict(mt, psums)
    nc.compile()
    return nc


def build_bc_program(n_rows, k_dim, m_dim, mb_cols=256, n_free=512, mode="fp16",
                     kb=4, wb_bufs=4):
    """Fallback one-core program: yt = (xt.T @ sign(w)).T + b, fp16 single pass."""
    f32 = mybir.dt.float32
    f32r = mybir.dt.float32r
    bf16 = mybir.dt.bfloat16
    fp16 = mybir.dt.float16
    if mode == "fp32r":
        mb_cols = min(mb_cols, P)
    if mode == "hilo":
        wb_bufs = 2
        kb = min(kb, 2)
    KT = k_dim // P
    kb = min(kb, KT)
    KB_N = KT // kb
    MTPB = mb_cols // P
    NMB = m_dim // mb_cols
    nchunks = (n_rows + n_free - 1) // n_free
    chunk_sz = [min(n_free, n_rows - j * n_free) for j in range(nchunks)]
    GRP = 1

    nc = bacc.Bacc()
    xt = nc.declare_dram_parameter("xt", [k_dim, n_rows], f32, isOutput=False)
    w = nc.declare_dram_parameter("w", [k_dim, m_dim], f32, isOutput=False)
    b = nc.declare_dram_parameter("b", [m_dim, 1], f32, isOutput=False)
    yt = nc.declare_dram_parameter("yt", [m_dim, n_rows], f32, isOutput=True)

    wb_dt = {"hilo": bf16, "fp32r": f32r, "fp16": fp16}[mode]
    xt_r = xt.ap().rearrange("(kt p) n -> p kt n", p=P)
    w_r = w.ap().rearrange("(kt p) m -> p kt m", p=P)
    b_r = b.ap().rearrange("(mt p) o -> p mt o", p=P)

    with TileContext(nc) as tc:
        with (
            tc.tile_pool(name="xstage", bufs=2) as xstage_pool,
            tc.tile_pool(name="xres", bufs=1) as xres_pool,
            tc.tile_pool(name="wstage", bufs=2 if mode == "fp16" else 3) as wstage_pool,
            tc.tile_pool(name="wbp", bufs=wb_bufs) as wb_pool,
            tc.tile_pool(name="biasp", bufs=1) as bias_pool,
            tc.tile_pool(name="outp", bufs=3) as out_pool,
            tc.tile_pool(
                name="psump",
                bufs=max(1, 8 // (GRP * MTPB * nchunks)),
                space="PSUM",
            ) as psum_pool,
        ):
            bts = bias_pool.tile([P, m_dim // P, 1], f32, name="bts", tag="bts")
            nc.sync.dma_start(out=bts[:], in_=b_r[:, :, :])

            def produce_wb_group(g):
                mbs = [g * GRP + i for i in range(GRP)]
                wbs = [
                    wb_pool.tile([P, KT * mb_cols], wb_dt, name="wb", tag="wb")
                    for _ in mbs
                ]
                for kg in range(KB_N):
                    for i, mb in enumerate(mbs):
                        wf = wstage_pool.tile(
                            [P, kb, mb_cols], f32, name="wf", tag="wf"
                        )
                        nc.sync.dma_start(
                            out=wf[:],
                            in_=w_r[:, kg * kb:(kg + 1) * kb,
                                    mb * mb_cols:(mb + 1) * mb_cols],
                        )
                        for t in range(kb):
                            k = kg * kb + t
                            nc.vector.tensor_scalar(
                                out=wbs[i][:, k * mb_cols:(k + 1) * mb_cols],
                                in0=wf[:, t, :],
                                scalar1=0.0,
                                scalar2=0.5,
                                op0=mybir.AluOpType.is_ge,
                                op1=mybir.AluOpType.subtract,
                            )
                return wbs

            NGRP = NMB // GRP
            wb_tiles = {0: produce_wb_group(0)}

            if mode == "hilo":
                xhi = xres_pool.tile([P, KT * n_rows], bf16, name="xhi", tag="xhi")
                xlo = xres_pool.tile([P, KT * n_rows], bf16, name="xlo", tag="xlo")
                for kg in range(KB_N):
                    xf = xstage_pool.tile([P, kb, n_rows], f32, name="xf", tag="xf")
                    nc.sync.dma_start(
                        out=xf[:], in_=xt_r[:, kg * kb:(kg + 1) * kb, :]
                    )
                    for t in range(kb):
                        k = kg * kb + t
                        hi = xhi[:, k * n_rows:(k + 1) * n_rows]
                        lo = xlo[:, k * n_rows:(k + 1) * n_rows]
                        nc.vector.tensor_copy(out=hi, in_=xf[:, t, :])
                        nc.vector.tensor_sub(out=lo, in0=xf[:, t, :], in1=hi)
            else:
                x_dt = f32r if mode == "fp32r" else fp16
                xall = xres_pool.tile([P, KT * n_rows], x_dt, name="xall", tag="xall")
                for kg in range(KB_N):
                    xf = xstage_pool.tile([P, kb, n_rows], f32, name="xf", tag="xf")
                    nc.sync.dma_start(
                        out=xf[:], in_=xt_r[:, kg * kb:(kg + 1) * kb, :]
                    )
                    for t in range(kb):
                        k = kg * kb + t
                        nc.scalar.copy(
                            out=xall[:, k * n_rows:(k + 1) * n_rows],
                            in_=xf[:, t, :],
                        )

            if NGRP > 1:
                wb_tiles[1] = produce_wb_group(1)

            for g in range(NGRP):
                wbs = wb_tiles.pop(g)
                if g + 1 < NGRP and g + 1 not in wb_tiles:
                    wb_tiles[g + 1] = produce_wb_group(g + 1)

                psums = [
                    [
                        psum_pool.tile(
                            [P, chunk_sz[j]], f32,
                            name=f"ps{mi}_{j}", tag=f"ps{mi}_{j}",
                        )
                        for j in range(nchunks)
                    ]
                    for mi in range(GRP * MTPB)
                ]
                for k in range(KT):
                    for mi in range(GRP * MTPB):
                        wb = wbs[mi // MTPB]
                        mw = mi % MTPB
                        lhsT = wb[:, k * mb_cols + mw * P:k * mb_cols + (mw + 1) * P]
                        if mode == "hilo":
                            for j in range(nchunks):
                                c0 = k * n_rows + j * n_free
                                rh = xhi[:, c0:c0 + chunk_sz[j]]
                                rl = xlo[:, c0:c0 + chunk_sz[j]]
                                nc.tensor.matmul(
                                    psums[mi][j][:], lhsT, rh,
                                    start=(k == 0), stop=False,
                                )
                                nc.tensor.matmul(
                                    psums[mi][j][:], lhsT, rl,
                                    start=False, stop=(k == KT - 1),
                                )
                        else:
                            for j in range(nchunks):
                                c0 = k * n_rows + j * n_free
                                rr = xall[:, c0:c0 + chunk_sz[j]]
                                nc.tensor.matmul(
                                    psums[mi][j][:], lhsT, rr,
                                    start=(k == 0), stop=(k == KT - 1),
                                )
                for mi in range(GRP * MTPB):
                    m = g * GRP * MTPB + mi
                    out_t = out_pool.tile([P, n_rows], f32, name="out_t", tag="out_t")
                    for j in range(nchunks):
                        nc.scalar.activation(
                            out=out_t[:, j * n_free:j * n_free + chunk_sz[j]],
                            in_=psums[mi][j][:],
                            func=mybir.ActivationFunctionType.Identity,
                            bias=bts[:, m, :],
                            scale=2.0,
                        )
                    nc.sync.dma_start(out=yt[m * P:(m + 1) * P, :], in_=out_t[:])
    nc.compile()
    return nc


_NC_CACHE = {}


def _get_program(mode=None):
    if mode is None:
        mode = MODE
    key = (N_FULL // N_CORES, K_DIM, M_DIM, mode)
    if key not in _NC_CACHE:
        if mode == "dr":
            _NC_CACHE[key] = build_dr_program(*key[:3])
        else:
            _NC_CACHE[key] = build_bc_program(*key[:3], mode=mode)
    return _NC_CACHE[key]


def _pack_w_dr(w):
    """sign(W) -> e4m3 bytes packed [P, MT, KP, 2, P] (per-m-tile contiguous)."""
    KT = K_DIM // P
    s8 = np.where(np.asarray(w, dtype=np.float32) >= 0, 0x38, 0xB8).astype(np.uint8)
    # k = (g*2 + s)*P + p, m = mt*P + mc
    s8 = s8.reshape(KT // 2, 2, P, M_DIM // P, P)       # [g, s, p, mt, mc]
    s8 = np.ascontiguousarray(s8.transpose(2, 3, 0, 1, 4))  # [p, mt, g, s, mc]
    return s8.view(FP8)


def _pack_x_dr(shard):
    """x shard [n, K] f32 -> (xhi [P, KT, n], xlo [P, L_LO, n]) e4m3."""
    n = shard.shape[0]
    KT = K_DIM // P
    hi = shard.astype(FP8)
    lo = (shard - hi.astype(np.float32))[:, :L_LO * P].astype(FP8)

    def to_tiles(a, kt):
        # [n, kt*P] -> [P, kt, n]
        return np.ascontiguousarray(
            a.T.reshape(kt, P, n).transpose(1, 0, 2)
        )

    return to_tiles(hi, KT), to_tiles(lo, L_LO)


def make_in_maps(x, w, b, mode=None):
    if mode is None:
        mode = MODE
    rows = x.shape[0] // N_CORES
    b = np.ascontiguousarray(np.asarray(b, dtype=np.float32).reshape(-1, 1))
    in_maps = []
    if mode == "dr":
        wpk = _pack_w_dr(w)
        for c in range(N_CORES):
            shard = np.asarray(x[c * rows:(c + 1) * rows, :], dtype=np.float32)
            xhi, xlo = _pack_x_dr(shard)
            in_maps.append({"xhi": xhi, "xlo": xlo, "wpk": wpk, "b": b})
    else:
        w = np.ascontiguousarray(np.asarray(w, dtype=np.float32))
        for c in range(N_CORES):
            shard = np.ascontiguousarray(
                np.asarray(x[c * rows:(c + 1) * rows, :], dtype=np.float32).T
            )
            in_maps.append({"xt": shard, "w": w, "b": b})
    return in_maps


def assemble_output(results, n_full=N_FULL, m_dim=M_DIM):
    rows = n_full // N_CORES
    y = np.empty((n_full, m_dim), dtype=np.float32)
    for c in range(N_CORES):
        y[c * rows:(c + 1) * rows, :] = results[c]["yt"].T
    return y


def kernel(x, kernel, bias):
    nc = _get_program()
    in_maps = make_in_maps(x, kernel, bias)
    res = run_bass_kernel_spmd(nc, in_maps, list(range(N_CORES)))
    return assemble_output(res.results)
